# revision 4
# baseline (speedup 1.0000x reference)
"""KGCapsuleTransformer TRN2 kernel (8 NeuronCores, SPMD over sequence rows).

Math (reference):
  q = seq @ Wq.T ; keys = kn @ Wk.T ; scores = q @ keys.T / sqrt(D)
  attn = softmax(scores); topw, topi = top_k(attn, 8)
  agg = sum_k topw * vals[topi],  vals = kn @ Wv.T
  gate = sigmoid([seq, agg] @ Wg.T + bg)
  enhanced = gate*agg + (1-gate)*seq
  out = LN(enhanced @ Wo.T + bo) * ln_g + ln_b + seq
  usage = scatter_add(topw at topi)

Device strategy per core (1024 of 8192 rows):
  - Fold M = Wq.T @ Wk / sqrt(D) (host, fp64). scores = seq @ M @ kn.T.
  - All score-chain matmuls in 3-pass fp16 hi/lo splits (error ~1e-8 in
    true-score units, below fp32 reorder noise) so top-8 selection matches
    the fp32 reference.
  - Scores never materialize: per 512-chunk, consume PSUM with
    DVE max8/max_index (candidates) + ACT exp(accum) for the softmax Z.
  - Merge 320 candidates/row via max8 + value-equality select.
  - agg via linearity: gbar = sum_k w_k * kn[topi_k] (gathered rows),
    agg = gbar @ Wv.T. Epilogue (gate/Wo/LN) in feature-major layout with
    PE transposes at the boundaries.
  - usage is assembled on host from returned (topi, topw): exact scatter-add.
"""

import numpy as np

import concourse.bass as bass
import concourse.tile as tile
import concourse.mybir as mybir
from concourse import bacc
from concourse.bass_utils import run_bass_kernel_spmd
from concourse.masks import make_identity

F32 = mybir.dt.float32
F16 = mybir.dt.float16
U16 = mybir.dt.uint16
U32 = mybir.dt.uint32
AF = mybir.ActivationFunctionType
OP = mybir.AluOpType
AX = mybir.AxisListType

B, S, D, N, K = 4, 2048, 1024, 20000, 8
LN_EPS = 1e-5
NCORES = 8
R = B * S                 # 8192 rows
RC = R // NCORES          # 1024 rows per core
NRT = RC // 128           # 8 row tiles per core
CH = 512                  # score chunk width
NCH = (N + CH - 1) // CH  # 40 chunks; last has 32 valid cols
NPAD = NCH * CH           # 20480
NCAND = NCH * 8           # 320
IDX_OFF = 1 << 20         # offset for the min-match index trick

_cache = {}


def _split16(x):
    hi = x.astype(np.float16)
    lo = (x - hi.astype(np.float32)).astype(np.float16)
    return np.ascontiguousarray(hi), np.ascontiguousarray(lo)


def _build():
    nc = bacc.Bacc("TRN2", target_bir_lowering=False, debug=False)

    def din(name, shape, dt):
        return nc.dram_tensor(name, shape, dt, kind="ExternalInput").ap()

    def dout(name, shape, dt):
        return nc.dram_tensor(name, shape, dt, kind="ExternalOutput").ap()

    mh_d = din("mh", [D, D], F16)        # M' = Wq.T@Wk/sqrt(D) * 1024, hi
    ml_d = din("ml", [D, D], F16)
    sh_d = din("sh", [D, RC], F16)       # seqT (this core's rows), hi
    sl_d = din("sl", [D, RC], F16)
    knh_d = din("knh", [D, NPAD], F16)   # knT * 64, hi (padded cols = 0? no: -inf pad below)
    knl_d = din("knl", [D, NPAD], F16)
    knrow_d = din("knrow", [N, D], F16)  # knowledge rows (gather table)
    wvt_d = din("wvt", [D, D], F16)      # Wv.T
    wgt_d = din("wgt", [2 * D, D], F16)  # Wg.T
    wot_d = din("wot", [D, D], F16)      # Wo.T
    bg_d = din("bg", [128, 8], F32)      # bg tiled per partition
    bo_d = din("bo", [128, 8], F32)
    lng_d = din("lng", [128, D], F32)    # ln_g replicated
    slnb_d = din("slnb", [RC, D], F32)   # seq rows + ln_b
    cbase_d = din("cbase", [128, NCAND], F32)  # chunk*CH - IDX_OFF per cand slot
    mpad_d = din("mpad", [128, CH], F32)  # 0 for valid col, -1e9 for padded col (last chunk)

    out_d = dout("out", [RC, D], F32)
    topi_d = dout("topi", [RC, K], U32)
    topw_d = dout("topw", [RC, K], F32)

    with tile.TileContext(nc) as tc:
        with (
            tc.tile_pool(name="pers", bufs=1) as pers,
            tc.tile_pool(name="cand", bufs=1) as candp,
        ):
            ident = pers.tile([128, 128], F32)
            make_identity(nc, ident[:])
            sh_t = pers.tile([128, 8, RC], F16)
            qkh_t = pers.tile([128, 8, RC], F16)
            qkl_t = pers.tile([128, 8, RC], F16)
            lng_t = pers.tile([128, D], F32)
            nc.sync.dma_start(lng_t[:], lng_d[:])
            bg_t = pers.tile([128, 8], F32)
            nc.sync.dma_start(bg_t[:], bg_d[:])
            bo_t = pers.tile([128, 8], F32)
            nc.sync.dma_start(bo_t[:], bo_d[:])
            cbase_t = pers.tile([128, NCAND], F32)
            nc.sync.dma_start(cbase_t[:], cbase_d[:])
            mpad_t = pers.tile([128, CH], F32)
            nc.sync.dma_start(mpad_t[:], mpad_d[:])
            eps_t = pers.tile([128, 1], F32)
            nc.vector.memset(eps_t[:], LN_EPS)

            cand_v = candp.tile([128, NRT, NCAND], F32)
            cand_i = candp.tile([128, NRT, NCAND], U16)
            zbuf = candp.tile([128, NRT, NCH], F32)

            # ---------- Phase A: qkT = (M')^T-contraction with seqT ----------
            with (
                tc.tile_pool(name="wa", bufs=1) as wa,
                tc.tile_pool(name="psa", bufs=2, space="PSUM") as psa,
            ):
                mh_t = wa.tile([128, 8, D], F16)
                ml_t = wa.tile([128, 8, D], F16)
                sl_t = wa.tile([128, 8, RC], F16)
                for dt in range(8):
                    nc.sync.dma_start(mh_t[:, dt], mh_d[bass.ts(dt, 128), :])
                    nc.sync.dma_start(ml_t[:, dt], ml_d[bass.ts(dt, 128), :])
                    nc.sync.dma_start(sh_t[:, dt], sh_d[bass.ts(dt, 128), :])
                    nc.sync.dma_start(sl_t[:, dt], sl_d[bass.ts(dt, 128), :])
                for et in range(8):
                    for rh in range(2):
                        ps = psa.tile([128, 512], F32, tag="psa")
                        n_mm = 0
                        for dt in range(8):
                            for (a, b_) in (
                                (mh_t, sh_t), (ml_t, sh_t), (mh_t, sl_t)
                            ):
                                nc.tensor.matmul(
                                    ps[:],
                                    a[:, dt, bass.ts(et, 128)],
                                    b_[:, dt, bass.ts(rh, 512)],
                                    start=(n_mm == 0), stop=(n_mm == 23),
                                )
                                n_mm += 1
                        dst = (et, slice(rh * 512, (rh + 1) * 512))
                        nc.vector.tensor_copy(qkh_t[:, dst[0], dst[1]], ps[:])
                        nc.vector.tensor_tensor(
                            out=qkl_t[:, dst[0], dst[1]],
                            in0=ps[:], in1=qkh_t[:, dst[0], dst[1]],
                            op=OP.subtract,
                        )

            # ---------- Phase B: scores chunks -> candidates + Z ----------
            with (
                tc.tile_pool(name="kc", bufs=2) as kc,
                tc.tile_pool(name="psb", bufs=4, space="PSUM") as psb,
                tc.tile_pool(name="expp", bufs=2) as expp,
            ):
                for ch in range(NCH):
                    knh_t = kc.tile([128, 8, CH], F16, tag="knh")
                    knl_t = kc.tile([128, 8, CH], F16, tag="knl")
                    for et in range(8):
                        nc.sync.dma_start(
                            knh_t[:, et], knh_d[bass.ts(et, 128), bass.ts(ch, CH)])
                        nc.sync.dma_start(
                            knl_t[:, et], knl_d[bass.ts(et, 128), bass.ts(ch, CH)])
                    for rt in range(NRT):
                        ps = psb.tile([128, CH], F32, tag="psb")
                        n_mm = 0
                        for et in range(8):
                            for (a, b_) in (
                                (qkh_t, knh_t), (qkl_t, knh_t), (qkh_t, knl_t)
                            ):
                                nc.tensor.matmul(
                                    ps[:],
                                    a[:, et, bass.ts(rt, 128)],
                                    b_[:, et],
                                    start=(n_mm == 0), stop=(n_mm == 23),
                                )
                                n_mm += 1
                        if ch == NCH - 1:
                            # mask padded columns to -1e9 before max/exp
                            nc.vector.tensor_add(ps[:], ps[:], mpad_t[:])
                        nc.vector.max(
                            out=cand_v[:, rt, ch * 8:(ch + 1) * 8], in_=ps[:])
                        nc.vector.max_index(
                            out=cand_i[:, rt, ch * 8:(ch + 1) * 8],
                            in_max=cand_v[:, rt, ch * 8:(ch + 1) * 8],
                            in_values=ps[:])
                        ex = expp.tile([128, CH], F16, tag="ex")
                        nc.scalar.activation(
                            ex[:], ps[:], AF.Exp, scale=1.0 / 65536.0,
                            accum_out=zbuf[:, rt, ch:ch + 1])

            # ---------- Phase C: merge + epilogue per row tile ----------
            with (
                tc.tile_pool(name="wc", bufs=1) as wc,
                tc.tile_pool(name="cc", bufs=1) as cc,
                tc.tile_pool(name="psc", bufs=2, space="PSUM") as psc,
            ):
                wvt_t = wc.tile([128, 8, D], F16)
                wot_t = wc.tile([128, 8, D], F16)
                wgt_t = wc.tile([128, 16, D], F16)
                for et in range(8):
                    nc.sync.dma_start(wvt_t[:, et], wvt_d[bass.ts(et, 128), :])
                    nc.sync.dma_start(wot_t[:, et], wot_d[bass.ts(et, 128), :])
                for ct in range(16):
                    nc.sync.dma_start(wgt_t[:, ct], wgt_d[bass.ts(ct, 128), :])

                for rt in range(NRT):
                    # softmax normalizer
                    z_t = cc.tile([128, 1], F32, tag="z")
                    nc.vector.tensor_reduce(
                        out=z_t[:], in_=zbuf[:, rt], op=OP.add, axis=AX.X)
                    rz_t = cc.tile([128, 1], F32, tag="rz")
                    nc.vector.reciprocal(rz_t[:], z_t[:])

                    # global top8 + indices via value match
                    top8 = cc.tile([128, 8], F32, tag="top8")
                    nc.vector.max(out=top8[:], in_=cand_v[:, rt])
                    cif = cc.tile([128, NCAND], F32, tag="cif")
                    nc.vector.tensor_copy(cif[:], cand_i[:, rt])
                    nc.vector.tensor_add(cif[:], cif[:], cbase_t[:])
                    sel = cc.tile([128, 8], F32, tag="sel")
                    for k in range(K):
                        m = cc.tile([128, NCAND], F32, tag="m")
                        nc.vector.scalar_tensor_tensor(
                            out=m[:], in0=cand_v[:, rt],
                            scalar=top8[:, k:k + 1], in1=cif[:],
                            op0=OP.is_equal, op1=OP.mult)
                        nc.vector.tensor_reduce(
                            out=sel[:, k:k + 1], in_=m[:], op=OP.min, axis=AX.X)
                    nc.vector.tensor_scalar_add(sel[:], sel[:], float(IDX_OFF))
                    seli = cc.tile([128, 8], U32, tag="seli")
                    nc.vector.tensor_copy(seli[:], sel[:])
                    nc.sync.dma_start(topi_d[bass.ts(rt, 128), :], seli[:])

                    # top weights
                    w8 = cc.tile([128, 8], F32, tag="w8")
                    nc.scalar.activation(w8[:], top8[:], AF.Exp, scale=1.0 / 65536.0)
                    nc.vector.tensor_scalar_mul(w8[:], w8[:], rz_t[:, 0:1])
                    nc.sync.dma_start(topw_d[bass.ts(rt, 128), :], w8[:])

                    # gbar = sum_k w_k * kn[topi_k]
                    gbar = cc.tile([128, D], F32, tag="gbar")
                    nc.vector.memset(gbar[:], 0.0)
                    for k in range(K):
                        g_t = cc.tile([128, D], F16, tag="g")
                        nc.gpsimd.indirect_dma_start(
                            out=g_t[:], out_offset=None, in_=knrow_d[:],
                            in_offset=bass.IndirectOffsetOnAxis(
                                ap=seli[:, k:k + 1], axis=0))
                        nc.vector.scalar_tensor_tensor(
                            out=gbar[:], in0=g_t[:], scalar=w8[:, k:k + 1],
                            in1=gbar[:], op0=OP.mult, op1=OP.add)

                    # gbarT via PE transpose
                    gbarT = cc.tile([128, 8, 128], F16, tag="gbarT")
                    for et in range(8):
                        pt = psc.tile([128, 128], F32, tag="pc")
                        nc.tensor.transpose(
                            pt[:], gbar[:, bass.ts(et, 128)], ident[:])
                        nc.vector.tensor_copy(gbarT[:, et], pt[:])

                    # aggT = Wv @ gbarT
                    aggT = cc.tile([128, 8, 128], F16, tag="aggT")
                    for dt in range(8):
                        pa = psc.tile([128, 128], F32, tag="pc")
                        for et in range(8):
                            nc.tensor.matmul(
                                pa[:], wvt_t[:, et, bass.ts(dt, 128)],
                                gbarT[:, et], start=(et == 0), stop=(et == 7))
                        nc.scalar.copy(aggT[:, dt], pa[:])

                    # gateT = sigmoid(Wg @ [seqT; aggT] + bg)
                    gateT = cc.tile([128, 8, 128], F16, tag="gateT")
                    for gt in range(8):
                        pg = psc.tile([128, 128], F32, tag="pc")
                        for ct in range(16):
                            rhs = (sh_t[:, ct, bass.ts(rt, 128)] if ct < 8
                                   else aggT[:, ct - 8])
                            nc.tensor.matmul(
                                pg[:], wgt_t[:, ct, bass.ts(gt, 128)], rhs,
                                start=(ct == 0), stop=(ct == 15))
                        nc.scalar.activation(
                            gateT[:, gt], pg[:], AF.Sigmoid,
                            bias=bg_t[:, gt:gt + 1])

                    # enhancedT = seqT + gateT*(aggT - seqT)
                    dif = cc.tile([128, 8, 128], F32, tag="dif")
                    nc.vector.tensor_tensor(
                        out=dif[:], in0=aggT[:],
                        in1=sh_t[:, :, bass.ts(rt, 128)], op=OP.subtract)
                    nc.vector.tensor_tensor(
                        out=dif[:], in0=dif[:], in1=gateT[:], op=OP.mult)
                    enhT = cc.tile([128, 8, 128], F16, tag="enhT")
                    nc.vector.tensor_tensor(
                        out=enhT[:], in0=dif[:],
                        in1=sh_t[:, :, bass.ts(rt, 128)], op=OP.add)

                    # hT = Wo @ enhancedT + bo
                    hT = cc.tile([128, 8, 128], F32, tag="hT")
                    for dt in range(8):
                        ph = psc.tile([128, 128], F32, tag="pc")
                        for et in range(8):
                            nc.tensor.matmul(
                                ph[:], wot_t[:, et, bass.ts(dt, 128)],
                                enhT[:, et], start=(et == 0), stop=(et == 7))
                        nc.vector.tensor_scalar(
                            out=hT[:, dt], in0=ph[:],
                            scalar1=bo_t[:, dt:dt + 1], scalar2=None,
                            op0=OP.add)

                    # back to row-major
                    h_t = cc.tile([128, D], F32, tag="h")
                    for dt in range(8):
                        pt2 = psc.tile([128, 128], F32, tag="pc")
                        nc.tensor.transpose(pt2[:], hT[:, dt], ident[:])
                        nc.scalar.copy(h_t[:, bass.ts(dt, 128)], pt2[:])

                    # layernorm + ln_g + (seq + ln_b)
                    red = cc.tile([128, 1], F32, tag="red")
                    nc.vector.tensor_reduce(
                        out=red[:], in_=h_t[:], op=OP.add, axis=AX.X)
                    mu = cc.tile([128, 1], F32, tag="mu")
                    nc.vector.tensor_scalar_mul(mu[:], red[:], 1.0 / D)
                    xc = cc.tile([128, D], F32, tag="xc")
                    nc.vector.tensor_scalar(
                        out=xc[:], in0=h_t[:], scalar1=mu[:, 0:1],
                        scalar2=None, op0=OP.subtract)
                    sq = cc.tile([128, D], F32, tag="sq")
                    nc.vector.tensor_tensor(
                        out=sq[:], in0=xc[:], in1=xc[:], op=OP.mult)
                    nc.vector.tensor_reduce(
                        out=red[:], in_=sq[:], op=OP.add, axis=AX.X)
                    std = cc.tile([128, 1], F32, tag="std")
                    nc.scalar.activation(
                        std[:], red[:], AF.Sqrt, scale=1.0 / D,
                        bias=eps_t[:, 0:1])
                    rstd = cc.tile([128, 1], F32, tag="rstd")
                    nc.vector.reciprocal(rstd[:], std[:])

                    slnb_t = cc.tile([128, D], F32, tag="slnb")
                    nc.sync.dma_start(slnb_t[:], slnb_d[bass.ts(rt, 128), :])
                    outn = cc.tile([128, D], F32, tag="outn")
                    nc.vector.scalar_tensor_tensor(
                        out=outn[:], in0=xc[:], scalar=rstd[:, 0:1],
                        in1=lng_t[:], op0=OP.mult, op1=OP.mult)
                    nc.vector.tensor_tensor(
                        out=outn[:], in0=outn[:], in1=slnb_t[:], op=OP.add)
                    nc.sync.dma_start(out_d[bass.ts(rt, 128), :], outn[:])

    nc.compile()
    return nc


def kernel(sequence, knowledge_embeddings, Wq, Wk, Wv, Wg, bg, Wo, bo, ln_g, ln_b):
    sequence = np.asarray(sequence, dtype=np.float32)
    kn = np.asarray(knowledge_embeddings, dtype=np.float32)
    Wq = np.asarray(Wq, dtype=np.float32)
    Wk = np.asarray(Wk, dtype=np.float32)
    Wv = np.asarray(Wv, dtype=np.float32)
    Wg = np.asarray(Wg, dtype=np.float32)
    bg = np.asarray(bg, dtype=np.float32)
    Wo = np.asarray(Wo, dtype=np.float32)
    bo = np.asarray(bo, dtype=np.float32)
    ln_g = np.asarray(ln_g, dtype=np.float32)
    ln_b = np.asarray(ln_b, dtype=np.float32)

    if "nc" not in _cache:
        _cache["nc"] = _build()
    nc = _cache["nc"]

    # ---- host prep ----
    seq2 = sequence.reshape(R, D)
    M = (Wq.T.astype(np.float64) @ Wk.astype(np.float64)) / np.sqrt(D)
    Mp = (M * 1024.0).astype(np.float32)
    mh, ml = _split16(Mp)
    seqT = np.ascontiguousarray(seq2.T)            # [D, R]
    shf, slf = _split16(seqT)
    knT = np.ascontiguousarray(kn.T) * 64.0        # [D, N]
    knTp = np.zeros((D, NPAD), dtype=np.float32)
    knTp[:, :N] = knT
    knh, knl = _split16(knTp)
    knrow = kn.astype(np.float16)
    wvt = np.ascontiguousarray(Wv.T).astype(np.float16)
    wgt = np.ascontiguousarray(Wg.T).astype(np.float16)
    wot = np.ascontiguousarray(Wo.T).astype(np.float16)
    bg_t = np.ascontiguousarray(bg.reshape(8, 128).T).astype(np.float32)
    bo_t = np.ascontiguousarray(bo.reshape(8, 128).T).astype(np.float32)
    lng = np.broadcast_to(ln_g, (128, D)).copy()
    slnb = seq2 + ln_b[None, :]
    cbase = np.broadcast_to(
        (np.repeat(np.arange(NCH) * CH, 8) - IDX_OFF).astype(np.float32),
        (128, NCAND)).copy()
    mpad = np.zeros((128, CH), dtype=np.float32)
    lastvalid = N - (NCH - 1) * CH
    mpad[:, lastvalid:] = -1e9

    in_maps = []
    for c in range(NCORES):
        rows = slice(c * RC, (c + 1) * RC)
        in_maps.append({
            "mh": mh, "ml": ml,
            "sh": np.ascontiguousarray(shf[:, rows]),
            "sl": np.ascontiguousarray(slf[:, rows]),
            "knh": knh, "knl": knl, "knrow": knrow,
            "wvt": wvt, "wgt": wgt, "wot": wot,
            "bg": bg_t, "bo": bo_t, "lng": lng,
            "slnb": np.ascontiguousarray(slnb[rows]),
            "cbase": cbase, "mpad": mpad,
        })

    res = run_bass_kernel_spmd(nc, in_maps, core_ids=list(range(NCORES)))
    _cache["last_res"] = res
    outs = res.results

    out = np.concatenate([r["out"] for r in outs], axis=0).reshape(B, S, D)
    topi = np.concatenate([r["topi"] for r in outs], axis=0)  # [R, K] uint32
    topw = np.concatenate([r["topw"] for r in outs], axis=0)  # [R, K] f32
    usage = np.zeros(N, dtype=np.float32)
    np.add.at(usage, np.minimum(topi.reshape(-1).astype(np.int64), N - 1),
              topw.reshape(-1))
    return out, usage


# revision 6
# speedup vs baseline: 1.0076x; 1.0076x over previous
"""KGCapsuleTransformer TRN2 kernel (8 NeuronCores, SPMD over sequence rows).

Math (reference):
  q = seq @ Wq.T ; keys = kn @ Wk.T ; scores = q @ keys.T / sqrt(D)
  attn = softmax(scores); topw, topi = top_k(attn, 8)
  agg = sum_k topw * vals[topi],  vals = kn @ Wv.T
  gate = sigmoid([seq, agg] @ Wg.T + bg)
  enhanced = gate*agg + (1-gate)*seq
  out = LN(enhanced @ Wo.T + bo) * ln_g + ln_b + seq
  usage = scatter_add(topw at topi)

Device strategy per core (1024 of 8192 rows):
  - Fold M = Wq.T @ Wk / sqrt(D) (host, fp64). scores = seq @ M @ kn.T.
  - All score-chain matmuls in 3-pass fp16 hi/lo splits (error ~1e-8 in
    true-score units, below fp32 reorder noise) so top-8 selection matches
    the fp32 reference.
  - Scores never materialize: per 512-chunk, consume PSUM with
    DVE max8/max_index (candidates) + ACT exp(accum) for the softmax Z.
  - Merge 320 candidates/row via max8 + value-equality select.
  - agg via linearity: gbar = sum_k w_k * kn[topi_k] (gathered rows),
    agg = gbar @ Wv.T. Epilogue (gate/Wo/LN) in feature-major layout with
    PE transposes at the boundaries.
  - usage is assembled on host from returned (topi, topw): exact scatter-add.
"""

import numpy as np

import concourse.bass as bass
import concourse.tile as tile
import concourse.mybir as mybir
from concourse import bacc
from concourse.bass_utils import run_bass_kernel_spmd
from concourse.masks import make_identity

F32 = mybir.dt.float32
F16 = mybir.dt.float16
U16 = mybir.dt.uint16
U32 = mybir.dt.uint32
AF = mybir.ActivationFunctionType
OP = mybir.AluOpType
AX = mybir.AxisListType

B, S, D, N, K = 4, 2048, 1024, 20000, 8
LN_EPS = 1e-5
NCORES = 8
R = B * S                 # 8192 rows
RC = R // NCORES          # 1024 rows per core
NRT = RC // 128           # 8 row tiles per core
CH = 512                  # score chunk width
NCH = (N + CH - 1) // CH  # 40 chunks; last has 32 valid cols
NPAD = NCH * CH           # 20480
NCAND = NCH * 8           # 320
IDX_OFF = 1 << 20         # offset for the min-match index trick

_cache = {}


def _split16(x):
    hi = x.astype(np.float16)
    lo = (x - hi.astype(np.float32)).astype(np.float16)
    return np.ascontiguousarray(hi), np.ascontiguousarray(lo)


def _build():
    nc = bacc.Bacc("TRN2", target_bir_lowering=False, debug=False)

    def din(name, shape, dt):
        return nc.dram_tensor(name, shape, dt, kind="ExternalInput").ap()

    def dout(name, shape, dt):
        return nc.dram_tensor(name, shape, dt, kind="ExternalOutput").ap()

    mh_d = din("mh", [D, D], F16)        # M' = Wq.T@Wk/sqrt(D) * 1024, hi
    ml_d = din("ml", [D, D], F16)
    sh_d = din("sh", [D, RC], F16)       # seqT (this core's rows), hi
    sl_d = din("sl", [D, RC], F16)
    knh_d = din("knh", [D, NPAD], F16)   # knT * 64, hi (padded cols = 0? no: -inf pad below)
    knl_d = din("knl", [D, NPAD], F16)
    knrow_d = din("knrow", [N, D], F16)  # knowledge rows (gather table)
    wvt_d = din("wvt", [D, D], F16)      # Wv.T
    wgt_d = din("wgt", [2 * D, D], F16)  # Wg.T
    wot_d = din("wot", [D, D], F16)      # Wo.T
    bg_d = din("bg", [128, 8], F32)      # bg tiled per partition
    bo_d = din("bo", [128, 8], F32)
    lng_d = din("lng", [128, D], F32)    # ln_g replicated
    slnb_d = din("slnb", [RC, D], F32)   # seq rows + ln_b
    cbase_d = din("cbase", [128, NCAND], F32)  # chunk*CH - IDX_OFF per cand slot
    mpad_d = din("mpad", [128, CH], F32)  # 0 for valid col, -1e9 for padded col (last chunk)

    out_d = dout("out", [RC, D], F32)
    topi_d = dout("topi", [RC, K], U32)
    topw_d = dout("topw", [RC, K], F32)

    with tile.TileContext(nc) as tc:
        with (
            tc.tile_pool(name="pers", bufs=1) as pers,
            tc.tile_pool(name="cand", bufs=1) as candp,
        ):
            ident = pers.tile([128, 128], F32)
            make_identity(nc, ident[:])
            sh_t = pers.tile([128, 8, RC], F16)
            qkh_t = pers.tile([128, 8, RC], F16)
            qkl_t = pers.tile([128, 8, RC], F16)
            lng_t = pers.tile([128, D], F32)
            nc.sync.dma_start(lng_t[:], lng_d[:])
            bg_t = pers.tile([128, 8], F32)
            nc.sync.dma_start(bg_t[:], bg_d[:])
            bo_t = pers.tile([128, 8], F32)
            nc.sync.dma_start(bo_t[:], bo_d[:])
            cbase_t = pers.tile([128, NCAND], F32)
            nc.sync.dma_start(cbase_t[:], cbase_d[:])
            mpad_t = pers.tile([128, CH], F32)
            nc.sync.dma_start(mpad_t[:], mpad_d[:])
            eps_t = pers.tile([128, 1], F32)
            nc.vector.memset(eps_t[:], LN_EPS)

            cand_v = candp.tile([128, NRT, NCAND], F32)
            cand_i = candp.tile([128, NRT, NCAND], U16)
            zbuf = candp.tile([128, NRT, NCH], F32)

            # ---------- Phase A: qkT = (M')^T-contraction with seqT ----------
            with (
                tc.tile_pool(name="wa", bufs=1) as wa,
                tc.tile_pool(name="psa", bufs=2, space="PSUM") as psa,
            ):
                mh_t = wa.tile([128, 8, D], F16)
                ml_t = wa.tile([128, 8, D], F16)
                sl_t = wa.tile([128, 8, RC], F16)
                for dt in range(8):
                    nc.sync.dma_start(mh_t[:, dt], mh_d[bass.ts(dt, 128), :])
                    nc.sync.dma_start(ml_t[:, dt], ml_d[bass.ts(dt, 128), :])
                    nc.sync.dma_start(sh_t[:, dt], sh_d[bass.ts(dt, 128), :])
                    nc.sync.dma_start(sl_t[:, dt], sl_d[bass.ts(dt, 128), :])
                for et in range(8):
                    for rh in range(2):
                        ps = psa.tile([128, 512], F32, tag="psa")
                        n_mm = 0
                        for dt in range(8):
                            for (a, b_) in (
                                (mh_t, sh_t), (ml_t, sh_t), (mh_t, sl_t)
                            ):
                                nc.tensor.matmul(
                                    ps[:],
                                    a[:, dt, bass.ts(et, 128)],
                                    b_[:, dt, bass.ts(rh, 512)],
                                    start=(n_mm == 0), stop=(n_mm == 23),
                                )
                                n_mm += 1
                        dst = (et, slice(rh * 512, (rh + 1) * 512))
                        nc.vector.tensor_copy(qkh_t[:, dst[0], dst[1]], ps[:])
                        nc.vector.tensor_tensor(
                            out=qkl_t[:, dst[0], dst[1]],
                            in0=ps[:], in1=qkh_t[:, dst[0], dst[1]],
                            op=OP.subtract,
                        )

            # ---------- Phase B: scores chunks -> candidates + Z ----------
            with (
                tc.tile_pool(name="kc", bufs=2) as kc,
                tc.tile_pool(name="psb", bufs=4, space="PSUM") as psb,
                tc.tile_pool(name="expp", bufs=2) as expp,
            ):
                for ch in range(NCH):
                    knh_t = kc.tile([128, 8, CH], F16, tag="knh")
                    knl_t = kc.tile([128, 8, CH], F16, tag="knl")
                    for et in range(8):
                        nc.sync.dma_start(
                            knh_t[:, et], knh_d[bass.ts(et, 128), bass.ts(ch, CH)])
                        nc.sync.dma_start(
                            knl_t[:, et], knl_d[bass.ts(et, 128), bass.ts(ch, CH)])
                    for rt in range(NRT):
                        ps = psb.tile([128, CH], F32, tag="psb")
                        n_mm = 0
                        for et in range(8):
                            for (a, b_) in (
                                (qkh_t, knh_t), (qkl_t, knh_t), (qkh_t, knl_t)
                            ):
                                nc.tensor.matmul(
                                    ps[:],
                                    a[:, et, bass.ts(rt, 128)],
                                    b_[:, et],
                                    start=(n_mm == 0), stop=(n_mm == 23),
                                )
                                n_mm += 1
                        if ch == NCH - 1:
                            # mask padded columns to -1e9 before max/exp
                            nc.vector.tensor_add(ps[:], ps[:], mpad_t[:])
                        nc.vector.max(
                            out=cand_v[:, rt, ch * 8:(ch + 1) * 8], in_=ps[:])
                        nc.vector.max_index(
                            out=cand_i[:, rt, ch * 8:(ch + 1) * 8],
                            in_max=cand_v[:, rt, ch * 8:(ch + 1) * 8],
                            in_values=ps[:])
                        ex = expp.tile([128, CH], F16, tag="ex")
                        nc.scalar.activation(
                            ex[:], ps[:], AF.Exp, scale=1.0 / 65536.0,
                            accum_out=zbuf[:, rt, ch:ch + 1])

            # ---------- Phase C: merge + epilogue per row tile ----------
            with (
                tc.tile_pool(name="wc", bufs=1) as wc,
                tc.tile_pool(name="cc", bufs=1) as cc,
                tc.tile_pool(name="psc", bufs=2, space="PSUM") as psc,
            ):
                wvt_t = wc.tile([128, 8, D], F16)
                wot_t = wc.tile([128, 8, D], F16)
                wgt_t = wc.tile([128, 16, D], F16)
                for et in range(8):
                    nc.sync.dma_start(wvt_t[:, et], wvt_d[bass.ts(et, 128), :])
                    nc.sync.dma_start(wot_t[:, et], wot_d[bass.ts(et, 128), :])
                for ct in range(16):
                    nc.sync.dma_start(wgt_t[:, ct], wgt_d[bass.ts(ct, 128), :])

                for rt in range(NRT):
                    # softmax normalizer
                    z_t = cc.tile([128, 1], F32, tag="z")
                    nc.vector.tensor_reduce(
                        out=z_t[:], in_=zbuf[:, rt], op=OP.add, axis=AX.X)
                    rz_t = cc.tile([128, 1], F32, tag="rz")
                    nc.vector.reciprocal(rz_t[:], z_t[:])

                    # global top8 + indices via value match
                    top8 = cc.tile([128, 8], F32, tag="top8")
                    nc.vector.max(out=top8[:], in_=cand_v[:, rt])
                    cif = cc.tile([128, NCAND], F32, tag="cif")
                    nc.vector.tensor_copy(cif[:], cand_i[:, rt])
                    nc.vector.tensor_add(cif[:], cif[:], cbase_t[:])
                    sel = cc.tile([128, 8], F32, tag="sel")
                    for k in range(K):
                        m = cc.tile([128, NCAND], F32, tag="m")
                        nc.vector.scalar_tensor_tensor(
                            out=m[:], in0=cand_v[:, rt],
                            scalar=top8[:, k:k + 1], in1=cif[:],
                            op0=OP.is_equal, op1=OP.mult)
                        nc.vector.tensor_reduce(
                            out=sel[:, k:k + 1], in_=m[:], op=OP.min, axis=AX.X)
                    nc.vector.tensor_scalar_add(sel[:], sel[:], float(IDX_OFF))
                    seli = cc.tile([128, 8], U32, tag="seli")
                    nc.vector.tensor_copy(seli[:], sel[:])
                    nc.sync.dma_start(topi_d[bass.ts(rt, 128), :], seli[:])

                    # top weights
                    w8 = cc.tile([128, 8], F32, tag="w8")
                    nc.scalar.activation(w8[:], top8[:], AF.Exp, scale=1.0 / 65536.0)
                    nc.vector.tensor_scalar_mul(w8[:], w8[:], rz_t[:, 0:1])
                    nc.sync.dma_start(topw_d[bass.ts(rt, 128), :], w8[:])

                    # gbar = sum_k w_k * kn[topi_k]
                    gbar = cc.tile([128, D], F32, tag="gbar")
                    nc.vector.memset(gbar[:], 0.0)
                    for k in range(K):
                        g_t = cc.tile([128, D], F16, tag="g")
                        nc.gpsimd.indirect_dma_start(
                            out=g_t[:], out_offset=None, in_=knrow_d[:],
                            in_offset=bass.IndirectOffsetOnAxis(
                                ap=seli[:, k:k + 1], axis=0))
                        nc.vector.scalar_tensor_tensor(
                            out=gbar[:], in0=g_t[:], scalar=w8[:, k:k + 1],
                            in1=gbar[:], op0=OP.mult, op1=OP.add)

                    # gbarT via PE transpose
                    gbarT = cc.tile([128, 8, 128], F16, tag="gbarT")
                    for et in range(8):
                        pt = psc.tile([128, 128], F32, tag="pc")
                        nc.tensor.transpose(
                            pt[:], gbar[:, bass.ts(et, 128)], ident[:])
                        nc.vector.tensor_copy(gbarT[:, et], pt[:])

                    # aggT = Wv @ gbarT
                    aggT = cc.tile([128, 8, 128], F16, tag="aggT")
                    for dt in range(8):
                        pa = psc.tile([128, 128], F32, tag="pc")
                        for et in range(8):
                            nc.tensor.matmul(
                                pa[:], wvt_t[:, et, bass.ts(dt, 128)],
                                gbarT[:, et], start=(et == 0), stop=(et == 7))
                        nc.scalar.copy(aggT[:, dt], pa[:])

                    # gateT = sigmoid(Wg @ [seqT; aggT] + bg)
                    gateT = cc.tile([128, 8, 128], F16, tag="gateT")
                    for gt in range(8):
                        pg = psc.tile([128, 128], F32, tag="pc")
                        for ct in range(16):
                            rhs = (sh_t[:, ct, bass.ts(rt, 128)] if ct < 8
                                   else aggT[:, ct - 8])
                            nc.tensor.matmul(
                                pg[:], wgt_t[:, ct, bass.ts(gt, 128)], rhs,
                                start=(ct == 0), stop=(ct == 15))
                        nc.scalar.activation(
                            gateT[:, gt], pg[:], AF.Sigmoid,
                            bias=bg_t[:, gt:gt + 1])

                    # enhancedT = seqT + gateT*(aggT - seqT)
                    dif = cc.tile([128, 8, 128], F32, tag="dif")
                    nc.vector.tensor_tensor(
                        out=dif[:], in0=aggT[:],
                        in1=sh_t[:, :, bass.ts(rt, 128)], op=OP.subtract)
                    nc.vector.tensor_tensor(
                        out=dif[:], in0=dif[:], in1=gateT[:], op=OP.mult)
                    enhT = cc.tile([128, 8, 128], F16, tag="enhT")
                    nc.vector.tensor_tensor(
                        out=enhT[:], in0=dif[:],
                        in1=sh_t[:, :, bass.ts(rt, 128)], op=OP.add)

                    # hT = Wo @ enhancedT + bo
                    hT = cc.tile([128, 8, 128], F32, tag="hT")
                    for dt in range(8):
                        ph = psc.tile([128, 128], F32, tag="pc")
                        for et in range(8):
                            nc.tensor.matmul(
                                ph[:], wot_t[:, et, bass.ts(dt, 128)],
                                enhT[:, et], start=(et == 0), stop=(et == 7))
                        nc.vector.tensor_scalar(
                            out=hT[:, dt], in0=ph[:],
                            scalar1=bo_t[:, dt:dt + 1], scalar2=None,
                            op0=OP.add)

                    # back to row-major
                    h_t = cc.tile([128, D], F32, tag="h")
                    for dt in range(8):
                        pt2 = psc.tile([128, 128], F32, tag="pc")
                        nc.tensor.transpose(pt2[:], hT[:, dt], ident[:])
                        nc.scalar.copy(h_t[:, bass.ts(dt, 128)], pt2[:])

                    # layernorm + ln_g + (seq + ln_b)
                    red = cc.tile([128, 1], F32, tag="red")
                    nc.vector.tensor_reduce(
                        out=red[:], in_=h_t[:], op=OP.add, axis=AX.X)
                    mu = cc.tile([128, 1], F32, tag="mu")
                    nc.vector.tensor_scalar_mul(mu[:], red[:], 1.0 / D)
                    xc = cc.tile([128, D], F32, tag="xc")
                    nc.vector.tensor_scalar(
                        out=xc[:], in0=h_t[:], scalar1=mu[:, 0:1],
                        scalar2=None, op0=OP.subtract)
                    sq = cc.tile([128, D], F32, tag="sq")
                    nc.vector.tensor_tensor(
                        out=sq[:], in0=xc[:], in1=xc[:], op=OP.mult)
                    nc.vector.tensor_reduce(
                        out=red[:], in_=sq[:], op=OP.add, axis=AX.X)
                    std = cc.tile([128, 1], F32, tag="std")
                    nc.scalar.activation(
                        std[:], red[:], AF.Sqrt, scale=1.0 / D,
                        bias=eps_t[:, 0:1])
                    rstd = cc.tile([128, 1], F32, tag="rstd")
                    nc.vector.reciprocal(rstd[:], std[:])

                    slnb_t = cc.tile([128, D], F32, tag="slnb")
                    nc.sync.dma_start(slnb_t[:], slnb_d[bass.ts(rt, 128), :])
                    outn = cc.tile([128, D], F32, tag="outn")
                    nc.vector.scalar_tensor_tensor(
                        out=outn[:], in0=xc[:], scalar=rstd[:, 0:1],
                        in1=lng_t[:], op0=OP.mult, op1=OP.mult)
                    nc.vector.tensor_tensor(
                        out=outn[:], in0=outn[:], in1=slnb_t[:], op=OP.add)
                    nc.sync.dma_start(out_d[bass.ts(rt, 128), :], outn[:])

    nc.compile()
    return nc


def kernel(sequence, knowledge_embeddings, Wq, Wk, Wv, Wg, bg, Wo, bo, ln_g, ln_b):
    sequence = np.asarray(sequence, dtype=np.float32)
    kn = np.asarray(knowledge_embeddings, dtype=np.float32)
    Wq = np.asarray(Wq, dtype=np.float32)
    Wk = np.asarray(Wk, dtype=np.float32)
    Wv = np.asarray(Wv, dtype=np.float32)
    Wg = np.asarray(Wg, dtype=np.float32)
    bg = np.asarray(bg, dtype=np.float32)
    Wo = np.asarray(Wo, dtype=np.float32)
    bo = np.asarray(bo, dtype=np.float32)
    ln_g = np.asarray(ln_g, dtype=np.float32)
    ln_b = np.asarray(ln_b, dtype=np.float32)

    if "nc" not in _cache:
        _cache["nc"] = _build()
    nc = _cache["nc"]

    # ---- host prep ----
    seq2 = sequence.reshape(R, D)
    M = (Wq.T.astype(np.float64) @ Wk.astype(np.float64)) / np.sqrt(D)
    Mp = (M * 1024.0).astype(np.float32)
    mh, ml = _split16(Mp)
    seqT = np.ascontiguousarray(seq2.T)            # [D, R]
    shf, slf = _split16(seqT)
    knT = np.ascontiguousarray(kn.T) * 64.0        # [D, N]
    knTp = np.zeros((D, NPAD), dtype=np.float32)
    knTp[:, :N] = knT
    knh, knl = _split16(knTp)
    knrow = kn.astype(np.float16)
    wvt = np.ascontiguousarray(Wv.T).astype(np.float16)
    wgt = np.ascontiguousarray(Wg.T).astype(np.float16)
    wot = np.ascontiguousarray(Wo.T).astype(np.float16)
    bg_t = np.ascontiguousarray(bg.reshape(8, 128).T).astype(np.float32)
    bo_t = np.ascontiguousarray(bo.reshape(8, 128).T).astype(np.float32)
    lng = np.broadcast_to(ln_g, (128, D)).copy()
    slnb = seq2 + ln_b[None, :]
    cbase = np.broadcast_to(
        (np.repeat(np.arange(NCH) * CH, 8) - IDX_OFF).astype(np.float32),
        (128, NCAND)).copy()
    mpad = np.zeros((128, CH), dtype=np.float32)
    lastvalid = N - (NCH - 1) * CH
    mpad[:, lastvalid:] = -1e9

    in_maps = []
    for c in range(NCORES):
        rows = slice(c * RC, (c + 1) * RC)
        in_maps.append({
            "mh": mh, "ml": ml,
            "sh": np.ascontiguousarray(shf[:, rows]),
            "sl": np.ascontiguousarray(slf[:, rows]),
            "knh": knh, "knl": knl, "knrow": knrow,
            "wvt": wvt, "wgt": wgt, "wot": wot,
            "bg": bg_t, "bo": bo_t, "lng": lng,
            "slnb": np.ascontiguousarray(slnb[rows]),
            "cbase": cbase, "mpad": mpad,
        })

    res = run_bass_kernel_spmd(nc, in_maps, core_ids=list(range(NCORES)))
    _cache["last_res"] = res
    outs = res.results

    out = np.concatenate([r["out"] for r in outs], axis=0).reshape(B, S, D)
    topi = np.concatenate([r["topi"] for r in outs], axis=0)  # [R, K] uint32
    topw = np.concatenate([r["topw"] for r in outs], axis=0)  # [R, K] f32
    usage = np.zeros(N, dtype=np.float32)
    np.add.at(usage, np.minimum(topi.reshape(-1).astype(np.int64), N - 1),
              topw.reshape(-1))
    return out, usage
